# revision 12
# baseline (speedup 1.0000x reference)
"""Trainium2 Bass kernel for a 2-branch, 2-layer GCN (nn_Net_7172595384447).

Strategy (8 NeuronCores, SPMD), v2:
  - Nodes sharded across cores by destination (6250 nodes/core, 49 tiles).
  - Self-loops NOT in the edge lists; added per dst tile with one
    transpose-accumulate matmul from an SBUF-resident copy of the shard.
  - Phase A: bf16 dense matmuls h1pre = [x@W1 | dat@Wd1] scaled by dinv ->
    hs1 table rows (bf16, 256B stride); AllGather -> full table.
  - Phase B: per dst tile, dma_gather incoming source rows; one-hot masks
    (iota/is_equal vs dloc) aggregate via operand-swapped PSUM matmuls:
    aggT[96,128] += msg[:,j,:96]^T @ W[:,j,:]; bias via K=1 matmul with
    sqrt(deg) row; h1sT = dinv^2 * relu(aggT) (dinv^2 row broadcast via K=1
    matmul); layer-2: hs2 = h1sT^T @ blockdiag(0.2W2,0.1Wd2) with branch-sum
    into 16 cols; AllGather hs2 table (16 used cols of 256B rows).
  - Phase C: same aggregation over hs2 rows (16-wide lhsT); transposed
    log_softmax: zT[16,128], exp on scalar engine, column sums via K=1
    matmul, single batched Ln at the end; output yT [16, SH], host
    transposes.
Host does graph preprocessing only (sharding, per-(core,tile) edge grouping
sorted by dst, degree counts, int16 biased gather-index tables).
"""

import numpy as np
import ml_dtypes

import concourse.bass as bass
import concourse.mybir as mybir
import concourse.tile as tile
from concourse import bacc
from concourse.bass_utils import run_bass_kernel_spmd
from concourse.masks import make_identity

NCORES = 8
N = 50000
FX = 512
FD = 64
SH = N // NCORES            # 6250 nodes per shard
TILES = (SH + 127) // 128   # 49 tiles (48 full + 106)
SH_PAD = TILES * 128        # 6272 padded shard rows
NT = SH_PAD * NCORES        # 50176 padded table rows
BIAS = 32768                # int16 index bias
H1 = 96                     # hs1 used cols (64 + 32)
H1P = 128                   # hs1 padded cols (256B rows)
H2 = 16                     # hs2 used cols (branch-summed)
H2P = 128                   # hs2 padded cols (256B rows)
R0 = 3200                   # local rows in AG chunk 0
PAD_DST = 300.0             # dst_local sentinel for pad slots

_CACHE = {}
BF16 = ml_dtypes.bfloat16


R0 = 3200                   # local rows in AG chunk 0 (25 tiles)
R1 = SH_PAD - R0            # 3072 rows in chunk 1


def _row_of_node(n):
    c = n // SH
    local = n % SH
    return np.where(local < R0, c * R0 + local,
                    NCORES * R0 + c * R1 + (local - R0))


def _host_prep(edge_index):
    src = np.asarray(edge_index[0], dtype=np.int64)
    dst = np.asarray(edge_index[1], dtype=np.int64)

    deg = (np.bincount(dst, minlength=N) + 1).astype(np.float64)
    dinv = (1.0 / np.sqrt(deg)).astype(np.float32)
    sqdeg = np.sqrt(deg).astype(np.float32)

    core_of = dst // SH
    tile_of = (dst % SH) // 128
    dloc_of = (dst % SH) % 128

    edge_rows = _row_of_node(src)
    per_ct = [[None] * TILES for _ in range(NCORES)]
    order = np.lexsort((tile_of, core_of))
    so_core = core_of[order]
    so_tile = tile_of[order]
    so_row = edge_rows[order]
    so_dloc = dloc_of[order]
    key = so_core * TILES + so_tile
    bounds = np.searchsorted(key, np.arange(NCORES * TILES + 1))
    for c in range(NCORES):
        for t in range(TILES):
            k = c * TILES + t
            sl = slice(bounds[k], bounds[k + 1])
            per_ct[c][t] = (so_row[sl], so_dloc[sl])

    # chunks per tile: global max over cores, always >= 1 trailing pad slot
    C_t = [max(len(per_ct[c][t][0]) // 128 + 1 for c in range(NCORES))
           for t in range(TILES)]

    tot_c = sum(C_t)
    idx_all = []
    dloc_all = []
    for c in range(NCORES):
        idx16 = np.zeros((16, tot_c * 8), dtype=np.int16)
        dloc = np.full((128, tot_c), PAD_DST, dtype=np.float32)
        off = 0
        for t in range(TILES):
            rows, dl = per_ct[c][t]
            ns = C_t[t] * 128
            e = len(rows)
            block = np.zeros(ns, dtype=np.int64)
            block[:e] = rows - BIAS
            block[e:] = 0
            idx16[:, off * 8:(off + C_t[t]) * 8] = (
                block.astype(np.int16).reshape(ns // 16, 16).T)
            dblock = np.full(ns, PAD_DST, dtype=np.float32)
            dblock[:e] = dl.astype(np.float32)
            dloc[:, off:off + C_t[t]] = dblock.reshape(C_t[t], 128).T
            off += C_t[t]
        idx_all.append(np.tile(idx16, (8, 1)).copy())
        dloc_all.append(dloc)

    return dinv, sqdeg, C_t, idx_all, dloc_all


def _build(C_t):
    dt = mybir.dt
    f32 = dt.float32
    bf = dt.bfloat16
    tot_c = sum(C_t)

    nc = bacc.Bacc("TRN2", target_bir_lowering=False, debug=False,
                   num_devices=NCORES, num_swdge_queues=4)
    xT = nc.dram_tensor("xT", [FX, SH], bf, kind="ExternalInput")
    datT = nc.dram_tensor("datT", [FD, SH], bf, kind="ExternalInput")
    W1 = nc.dram_tensor("W1", [FX, 64], bf, kind="ExternalInput")
    Wd1 = nc.dram_tensor("Wd1", [FD, 32], bf, kind="ExternalInput")
    W2b = nc.dram_tensor("W2b", [H1, H2], bf, kind="ExternalInput")
    brow = nc.dram_tensor("brow", [1, H1], bf, kind="ExternalInput")
    bcrow = nc.dram_tensor("bcrow", [1, H2], bf, kind="ExternalInput")
    dinv_t = nc.dram_tensor("dinv_t", [128, TILES], f32, kind="ExternalInput")
    rows_r = nc.dram_tensor("rows_r", [4, SH_PAD], bf, kind="ExternalInput")
    # rows_r rows: 0 = sqdeg, 1 = dinv, 2 = dinv^2, 3 = ones
    idx_d = nc.dram_tensor("idx", [128, tot_c * 8], dt.int16, kind="ExternalInput")
    dloc_d = nc.dram_tensor("dloc", [128, tot_c], f32, kind="ExternalInput")
    y = nc.dram_tensor("y", [16, SH], f32, kind="ExternalOutput")

    with tile.TileContext(nc) as tc:
        with tc.tile_pool(name="const", bufs=1) as constp, \
             tc.tile_pool(name="dram", bufs=1, space="DRAM") as dram:
            idx_t = constp.tile([128, tot_c * 8], dt.int16)
            nc.sync.dma_start(out=idx_t[:], in_=idx_d[:])
            dloc_t = constp.tile([128, tot_c], bf)
            nc.gpsimd.dma_start(out=dloc_t[:], in_=dloc_d[:])
            dinv_s = constp.tile([128, TILES], f32)
            nc.sync.dma_start(out=dinv_s[:], in_=dinv_t[:])
            rowpack = constp.tile([65, SH_PAD], bf)
            nc.sync.dma_start(out=rowpack[0:1, :], in_=rows_r[0:1, :])
            nc.sync.dma_start(out=rowpack[32:33, :], in_=rows_r[1:2, :])
            nc.sync.dma_start(out=rowpack[64:65, :], in_=rows_r[2:3, :])
            ones_sm = constp.tile([65, 128], bf)
            nc.vector.memset(ones_sm[:], 1.0)
            brow_s = constp.tile([1, H1], bf)
            nc.sync.dma_start(out=brow_s[:], in_=brow[:])
            bcrow_s = constp.tile([1, H2], bf)
            nc.sync.dma_start(out=bcrow_s[:], in_=bcrow[:])
            w2b_s = constp.tile([H1, H2], bf)
            nc.sync.dma_start(out=w2b_s[:], in_=W2b[:])
            ident = constp.tile([128, 128], bf)
            make_identity(nc, ident[:])
            iota_i = constp.tile([128, 128], dt.int32)
            nc.gpsimd.iota(iota_i[:], pattern=[[1, 128]], base=0,
                           channel_multiplier=0)
            iota_f = constp.tile([128, 128], bf)
            nc.vector.tensor_copy(iota_f[:], iota_i[:])
            ones_c = constp.tile([16, 1], bf)
            nc.vector.memset(ones_c[:], 1.0)

            hs1_sb = constp.tile([128, TILES, H1], bf)   # own shard rows
            nc.vector.memset(hs1_sb[:], 0.0)
            hs2_sb = constp.tile([128, TILES, H2], bf)
            nc.vector.memset(hs2_sb[:], 0.0)
            z_all = constp.tile([16, SH_PAD], bf)
            s_all = constp.tile([1, SH_PAD], bf)
            ls_all = constp.tile([1, SH_PAD], bf)

            hs1_shard = dram.tile([SH_PAD, H1P], bf)
            hs1_full = dram.tile([NT, H1P], bf)
            hs2_shard = dram.tile([SH_PAD, H2P], bf)
            hs2_full = dram.tile([NT, H2P], bf)

            # ---------------- Phase A ----------------
            with tc.tile_pool(name="phA", bufs=3) as pa, \
                 tc.tile_pool(name="phA_w", bufs=1) as paw, \
                 tc.tile_pool(name="psA", bufs=4, space="PSUM") as psa:
                w1_s = paw.tile([128, FX // 128, 64], bf)
                for k in range(FX // 128):
                    nc.sync.dma_start(out=w1_s[:, k, :],
                                      in_=W1[k * 128:(k + 1) * 128, :])
                wd1_s = paw.tile([FD, 32], bf)
                nc.sync.dma_start(out=wd1_s[:], in_=Wd1[:])
                xT_s = []
                for k in range(FX // 128):
                    st = paw.tile([128, SH], bf, tag=f"xT{k}")
                    nc.sync.dma_start(out=st[:],
                                      in_=xT[k * 128:(k + 1) * 128, :])
                    xT_s.append(st)
                datT_s = paw.tile([FD, SH], bf)
                nc.sync.dma_start(out=datT_s[:], in_=datT[:])

                for t in range(TILES):
                    nt = min(128, SH - t * 128)
                    ps = psa.tile([128, H1], f32, space="PSUM", tag="psA")
                    for k in range(FX // 128):
                        nc.tensor.matmul(
                            out=ps[:nt, 0:64],
                            lhsT=xT_s[k][:, t * 128:t * 128 + nt],
                            rhs=w1_s[:, k, :],
                            start=(k == 0), stop=(k == FX // 128 - 1))
                    nc.tensor.matmul(
                        out=ps[:nt, 64:96],
                        lhsT=datT_s[:, t * 128:t * 128 + nt],
                        rhs=wd1_s[:],
                        start=True, stop=True)
                    nc.vector.tensor_tensor(
                        out=hs1_sb[:nt, t, :], in0=ps[:nt, :],
                        in1=dinv_s[:nt, t:t + 1].to_broadcast([nt, H1]),
                        op=mybir.AluOpType.mult)
                    nc.sync.dma_start(
                        out=hs1_shard[t * 128:t * 128 + nt, 0:H1],
                        in_=hs1_sb[:nt, t, :])
                    if t == R0 // 128 - 1:
                        nc.gpsimd.collective_compute(
                            "AllGather", mybir.AluOpType.bypass,
                            replica_groups=[list(range(NCORES))],
                            ins=[hs1_shard[0:R0, :].opt()],
                            outs=[hs1_full[0:NCORES * R0, :].opt()])

            nc.gpsimd.collective_compute(
                "AllGather", mybir.AluOpType.bypass,
                replica_groups=[list(range(NCORES))],
                ins=[hs1_shard[R0:SH_PAD, :].opt()],
                outs=[hs1_full[NCORES * R0:NT, :].opt()])

            # ---------------- Phase B ----------------
            with tc.tile_pool(name="phB", bufs=8) as pb, \
                 tc.tile_pool(name="phBw", bufs=6) as pbw, \
                 tc.tile_pool(name="psB", bufs=2, space="PSUM") as psb, \
                 tc.tile_pool(name="psBd", bufs=2, space="PSUM") as psbd, \
                 tc.tile_pool(name="psB2", bufs=2, space="PSUM") as psb2:
                off = 0
                for t in range(TILES):
                    ct = C_t[t]
                    nt = min(128, SH - t * 128)
                    ni = ct * 128
                    tr = slice(t * 128, (t + 1) * 128)
                    msg = pb.tile([128, ct, H1P], bf, tag="msg")
                    nc.gpsimd.dma_gather(
                        out_ap=msg[:], in_ap=hs1_full[BIAS:, :],
                        idxs_ap=idx_t[:, off * 8:(off + ct) * 8],
                        num_idxs=ni, num_idxs_reg=ni, elem_size=H1P,
                        single_packet=False, queue_num=t % 4)
                    W = pbw.tile([128, ct, 128], bf, tag="W")
                    nc.any.tensor_tensor(
                        out=W[:],
                        in0=iota_f[:, None, :].to_broadcast([128, ct, 128]),
                        in1=dloc_t[:, off:off + ct, None].to_broadcast(
                            [128, ct, 128]),
                        op=mybir.AluOpType.is_equal)
                    psT = psb.tile([H1, 128], f32, space="PSUM", tag="psT")
                    for j in range(ct):
                        nc.tensor.matmul(
                            out=psT[:], lhsT=msg[:, j, :H1], rhs=W[:, j, :],
                            start=(j == 0), stop=False)
                    # self-loop rows (transpose-accumulate own shard tile)
                    nc.tensor.matmul(
                        out=psT[:], lhsT=hs1_sb[:, t, :], rhs=ident[:],
                        start=False, stop=False)
                    # bias row: aggT += brow^T @ sqdeg_row
                    nc.tensor.matmul(
                        out=psT[:], lhsT=brow_s[:], rhs=rowpack[0:1, tr],
                        start=False, stop=True)
                    # dinv^2 broadcast [H1, 128]
                    psD = psbd.tile([H1, 128], f32, space="PSUM", tag="psD")
                    nc.tensor.matmul(
                        out=psD[:], lhsT=ones_sm[64:65, 0:H1], rhs=rowpack[64:65, tr],
                        start=True, stop=True)
                    h1r = pb.tile([H1, 128], f32, tag="h1r")
                    nc.vector.tensor_scalar(
                        out=h1r[:], in0=psT[:], scalar1=0.0, scalar2=None,
                        op0=mybir.AluOpType.max)
                    h1sT = pb.tile([H1, 128], bf, tag="h1sT")
                    nc.vector.tensor_tensor(
                        out=h1sT[:], in0=h1r[:], in1=psD[:],
                        op=mybir.AluOpType.mult)
                    # layer 2 (branch-summed): hs2 = h1sT^T @ w2b
                    ps2 = psb2.tile([128, H2], f32, space="PSUM", tag="ps2")
                    nc.tensor.matmul(out=ps2[:], lhsT=h1sT[0:64, :],
                                     rhs=w2b_s[0:64, :], start=True, stop=False)
                    nc.tensor.matmul(out=ps2[:], lhsT=h1sT[64:96, :],
                                     rhs=w2b_s[64:96, :], start=False, stop=True)
                    nc.vector.tensor_copy(hs2_sb[:, t, :], ps2[:])
                    nc.sync.dma_start(
                        out=hs2_shard[t * 128:t * 128 + nt, 0:H2],
                        in_=hs2_sb[:nt, t, :])
                    if t == R0 // 128 - 1:
                        nc.gpsimd.collective_compute(
                            "AllGather", mybir.AluOpType.bypass,
                            replica_groups=[list(range(NCORES))],
                            ins=[hs2_shard[0:R0, :].opt()],
                            outs=[hs2_full[0:NCORES * R0, :].opt()])
                    off += ct

            nc.gpsimd.collective_compute(
                "AllGather", mybir.AluOpType.bypass,
                replica_groups=[list(range(NCORES))],
                ins=[hs2_shard[R0:SH_PAD, :].opt()],
                outs=[hs2_full[NCORES * R0:NT, :].opt()])

            # ---------------- Phase C ----------------
            with tc.tile_pool(name="phC", bufs=8) as pc_, \
                 tc.tile_pool(name="phCw", bufs=6) as pcw, \
                 tc.tile_pool(name="psC", bufs=2, space="PSUM") as psc, \
                 tc.tile_pool(name="psCd", bufs=2, space="PSUM") as pscd, \
                 tc.tile_pool(name="psCs", bufs=2, space="PSUM") as pscs:
                off = 0
                for t in range(TILES):
                    ct = C_t[t]
                    nt = min(128, SH - t * 128)
                    ni = ct * 128
                    tr = slice(t * 128, (t + 1) * 128)
                    msg = pc_.tile([128, ct, H2P], bf, tag="msg2")
                    nc.gpsimd.dma_gather(
                        out_ap=msg[:], in_ap=hs2_full[BIAS:, :],
                        idxs_ap=idx_t[:, off * 8:(off + ct) * 8],
                        num_idxs=ni, num_idxs_reg=ni, elem_size=H2P,
                        single_packet=False, queue_num=t % 4)
                    W = pcw.tile([128, ct, 128], bf, tag="W2")
                    nc.any.tensor_tensor(
                        out=W[:],
                        in0=iota_f[:, None, :].to_broadcast([128, ct, 128]),
                        in1=dloc_t[:, off:off + ct, None].to_broadcast(
                            [128, ct, 128]),
                        op=mybir.AluOpType.is_equal)
                    psC = psc.tile([H2, 128], f32, space="PSUM", tag="psC")
                    for j in range(ct):
                        nc.tensor.matmul(
                            out=psC[:], lhsT=msg[:, j, :H2], rhs=W[:, j, :],
                            start=(j == 0), stop=False)
                    nc.tensor.matmul(
                        out=psC[:], lhsT=hs2_sb[:, t, :], rhs=ident[:],
                        start=False, stop=False)
                    nc.tensor.matmul(
                        out=psC[:], lhsT=bcrow_s[:], rhs=rowpack[0:1, tr],
                        start=False, stop=True)
                    psD = pscd.tile([H2, 128], f32, space="PSUM", tag="psDC")
                    nc.tensor.matmul(
                        out=psD[:], lhsT=ones_sm[32:33, 0:H2], rhs=rowpack[32:33, tr],
                        start=True, stop=True)
                    dDC = pc_.tile([H2, 128], f32, tag="dDC")
                    nc.vector.tensor_copy(dDC[:], psD[:])
                    nc.vector.tensor_tensor(
                        out=z_all[:, tr], in0=psC[:], in1=dDC[:],
                        op=mybir.AluOpType.mult)
                    ex = pc_.tile([H2, 128], bf, tag="ex")
                    nc.scalar.activation(
                        ex[:], z_all[:, tr], mybir.ActivationFunctionType.Exp)
                    psS = pscs.tile([1, 128], f32, space="PSUM", tag="psS")
                    nc.tensor.matmul(
                        out=psS[:], lhsT=ones_c[:], rhs=ex[:],
                        start=True, stop=True)
                    nc.vector.tensor_copy(s_all[:, tr], psS[:])
                    if t == R0 // 128 - 1:
                        nc.scalar.activation(
                            ls_all[:, 0:R0], s_all[:, 0:R0],
                            mybir.ActivationFunctionType.Ln)
                        with tc.tile_pool(name="psEa", bufs=2,
                                          space="PSUM") as psea:
                            for g in range(0, R0 - 512, 512):
                                psL = psea.tile([H2, 512], f32, space="PSUM",
                                                tag="psLa")
                                nc.tensor.matmul(
                                    out=psL[:], lhsT=ones_sm[0:1, 0:H2],
                                    rhs=ls_all[0:1, g:g + 512],
                                    start=True, stop=True)
                                ot = pc_.tile([H2, 512], f32, tag="ota")
                                nc.vector.tensor_tensor(
                                    out=ot[:], in0=z_all[:, g:g + 512],
                                    in1=psL[:], op=mybir.AluOpType.subtract)
                                nc.sync.dma_start(out=y[:, g:g + 512],
                                                  in_=ot[:])
                    off += ct

                # batched epilogue: ls = ln(s); out = z - ls
                nc.scalar.activation(
                    ls_all[:, R0:SH_PAD], s_all[:, R0:SH_PAD],
                    mybir.ActivationFunctionType.Ln)
                NGRP = 512
                with tc.tile_pool(name="psE", bufs=2, space="PSUM") as pse:
                    for g in range(R0 - NGRP, SH, NGRP):
                        w = min(NGRP, SH - g)
                        psL = pse.tile([H2, NGRP], f32, space="PSUM", tag="psL")
                        nc.tensor.matmul(
                            out=psL[:, :w], lhsT=ones_sm[0:1, 0:H2],
                            rhs=ls_all[0:1, g:g + w], start=True, stop=True)
                        ot = pc_.tile([H2, NGRP], f32, tag="ot")
                        nc.vector.tensor_tensor(
                            out=ot[:, :w], in0=z_all[:, g:g + w],
                            in1=psL[:, :w], op=mybir.AluOpType.subtract)
                        nc.sync.dma_start(out=y[:, g:g + w], in_=ot[:, :w])

    nc.compile()
    return nc


def kernel(x, dat, edge_index, W1, b1, W2, b2, Wd1, bd1, Wd2, bd2):
    x = np.asarray(x, dtype=np.float32)
    dat = np.asarray(dat, dtype=np.float32)
    dinv, sqdeg, C_t, idx_all, dloc_all = _host_prep(np.asarray(edge_index))

    key = tuple(C_t)
    if key not in _CACHE:
        _CACHE[key] = _build(C_t)
    nc = _CACHE[key]

    W1f = np.asarray(W1, np.float32).astype(BF16)
    Wd1f = np.asarray(Wd1, np.float32).astype(BF16)
    w2b = np.concatenate([0.2 * np.asarray(W2, np.float32),
                          0.1 * np.asarray(Wd2, np.float32)], axis=0).astype(BF16)
    brow = np.concatenate([np.asarray(b1, np.float32),
                           np.asarray(bd1, np.float32)])[None, :].astype(BF16)
    bcrow = (0.2 * np.asarray(b2, np.float32) +
             0.1 * np.asarray(bd2, np.float32))[None, :].astype(BF16)

    in_maps = []
    for c in range(NCORES):
        lo, hi = c * SH, (c + 1) * SH
        dv = np.zeros((128, TILES), np.float32)
        dv_flat = dinv[lo:hi]
        dv[:, :TILES - 1] = dv_flat[:(TILES - 1) * 128].reshape(TILES - 1, 128).T
        rem = SH - (TILES - 1) * 128
        dv[:rem, TILES - 1] = dv_flat[(TILES - 1) * 128:]
        rows4 = np.zeros((4, SH_PAD), np.float32)
        rows4[0, :SH] = sqdeg[lo:hi]
        rows4[1, :SH] = dinv[lo:hi]
        rows4[2, :SH] = dinv[lo:hi] ** 2
        rows4[3, :] = 1.0
        in_maps.append({
            "xT": np.ascontiguousarray(x[lo:hi].T).astype(BF16),
            "datT": np.ascontiguousarray(dat[lo:hi].T).astype(BF16),
            "W1": W1f, "Wd1": Wd1f, "W2b": w2b,
            "brow": brow, "bcrow": bcrow,
            "dinv_t": dv, "rows_r": rows4.astype(BF16),
            "idx": idx_all[c], "dloc": dloc_all[c],
        })

    res = run_bass_kernel_spmd(nc, in_maps, core_ids=list(range(NCORES)))
    out = np.concatenate(
        [np.asarray(res.results[c]["y"]).T for c in range(NCORES)], axis=0)
    return out.astype(np.float32)


# revision 13
# speedup vs baseline: 1.1225x; 1.1225x over previous
"""Trainium2 Bass kernel for a 2-branch, 2-layer GCN (nn_Net_7172595384447).

Strategy (8 NeuronCores, SPMD), v2:
  - Nodes sharded across cores by destination (6250 nodes/core, 49 tiles).
  - Self-loops NOT in the edge lists; added per dst tile with one
    transpose-accumulate matmul from an SBUF-resident copy of the shard.
  - Phase A: bf16 dense matmuls h1pre = [x@W1 | dat@Wd1] scaled by dinv ->
    hs1 table rows (bf16, 256B stride); AllGather -> full table.
  - Phase B: per dst tile, dma_gather incoming source rows; one-hot masks
    (iota/is_equal vs dloc) aggregate via operand-swapped PSUM matmuls:
    aggT[96,128] += msg[:,j,:96]^T @ W[:,j,:]; bias via K=1 matmul with
    sqrt(deg) row; h1sT = dinv^2 * relu(aggT) (dinv^2 row broadcast via K=1
    matmul); layer-2: hs2 = h1sT^T @ blockdiag(0.2W2,0.1Wd2) with branch-sum
    into 16 cols; AllGather hs2 table (16 used cols of 256B rows).
  - Phase C: same aggregation over hs2 rows (16-wide lhsT); transposed
    log_softmax: zT[16,128], exp on scalar engine, column sums via K=1
    matmul, single batched Ln at the end; output yT [16, SH], host
    transposes.
Host does graph preprocessing only (sharding, per-(core,tile) edge grouping
sorted by dst, degree counts, int16 biased gather-index tables).
"""

import numpy as np
import ml_dtypes

import concourse.bass as bass
import concourse.mybir as mybir
import concourse.tile as tile
from concourse import bacc
from concourse.bass_utils import run_bass_kernel_spmd
from concourse.masks import make_identity

NCORES = 8
N = 50000
FX = 512
FD = 64
SH = N // NCORES            # 6250 nodes per shard
TILES = (SH + 127) // 128   # 49 tiles (48 full + 106)
SH_PAD = TILES * 128        # 6272 padded shard rows
NT = SH_PAD * NCORES        # 50176 padded table rows
BIAS = 32768                # int16 index bias
H1 = 96                     # hs1 used cols (64 + 32)
H1P = 128                   # hs1 padded cols (256B rows)
H2 = 16                     # hs2 used cols (branch-summed)
H2P = 128                   # hs2 padded cols (256B rows)
R0 = 3200                   # local rows in AG chunk 0
PAD_DST = 300.0             # dst_local sentinel for pad slots

_CACHE = {}
BF16 = ml_dtypes.bfloat16


R0 = 3200                   # local rows in AG chunk 0 (25 tiles)
R1 = SH_PAD - R0            # 3072 rows in chunk 1


def _row_of_node(n):
    c = n // SH
    local = n % SH
    return np.where(local < R0, c * R0 + local,
                    NCORES * R0 + c * R1 + (local - R0))


def _host_prep(edge_index):
    src = np.asarray(edge_index[0], dtype=np.int64)
    dst = np.asarray(edge_index[1], dtype=np.int64)

    deg = (np.bincount(dst, minlength=N) + 1).astype(np.float64)
    dinv = (1.0 / np.sqrt(deg)).astype(np.float32)
    sqdeg = np.sqrt(deg).astype(np.float32)

    core_of = dst // SH
    tile_of = (dst % SH) // 128
    dloc_of = (dst % SH) % 128

    edge_rows = _row_of_node(src)
    per_ct = [[None] * TILES for _ in range(NCORES)]
    order = np.lexsort((tile_of, core_of))
    so_core = core_of[order]
    so_tile = tile_of[order]
    so_row = edge_rows[order]
    so_dloc = dloc_of[order]
    key = so_core * TILES + so_tile
    bounds = np.searchsorted(key, np.arange(NCORES * TILES + 1))
    for c in range(NCORES):
        for t in range(TILES):
            k = c * TILES + t
            sl = slice(bounds[k], bounds[k + 1])
            per_ct[c][t] = (so_row[sl], so_dloc[sl])

    # chunks per tile: global max over cores, always >= 1 trailing pad slot
    C_t = [max(len(per_ct[c][t][0]) // 128 + 1 for c in range(NCORES))
           for t in range(TILES)]

    tot_c = sum(C_t)
    idx_all = []
    dloc_all = []
    for c in range(NCORES):
        idx16 = np.zeros((16, tot_c * 8), dtype=np.int16)
        dloc = np.full((128, tot_c), PAD_DST, dtype=np.float32)
        off = 0
        for t in range(TILES):
            rows, dl = per_ct[c][t]
            ns = C_t[t] * 128
            e = len(rows)
            block = np.zeros(ns, dtype=np.int64)
            block[:e] = rows - BIAS
            block[e:] = 0
            idx16[:, off * 8:(off + C_t[t]) * 8] = (
                block.astype(np.int16).reshape(ns // 16, 16).T)
            dblock = np.full(ns, PAD_DST, dtype=np.float32)
            dblock[:e] = dl.astype(np.float32)
            dloc[:, off:off + C_t[t]] = dblock.reshape(C_t[t], 128).T
            off += C_t[t]
        idx_all.append(np.tile(idx16, (8, 1)).copy())
        dloc_all.append(dloc)

    return dinv, sqdeg, C_t, idx_all, dloc_all


def _build(C_t):
    dt = mybir.dt
    f32 = dt.float32
    bf = dt.bfloat16
    tot_c = sum(C_t)

    nc = bacc.Bacc("TRN2", target_bir_lowering=False, debug=False,
                   num_devices=NCORES, num_swdge_queues=4)
    xT = nc.dram_tensor("xT", [FX, SH], bf, kind="ExternalInput")
    datT = nc.dram_tensor("datT", [FD, SH], bf, kind="ExternalInput")
    W1 = nc.dram_tensor("W1", [FX, 64], bf, kind="ExternalInput")
    Wd1 = nc.dram_tensor("Wd1", [FD, 32], bf, kind="ExternalInput")
    W2b = nc.dram_tensor("W2b", [H1, H2], bf, kind="ExternalInput")
    brow = nc.dram_tensor("brow", [1, H1], bf, kind="ExternalInput")
    bcrow = nc.dram_tensor("bcrow", [1, H2], bf, kind="ExternalInput")
    dinv_t = nc.dram_tensor("dinv_t", [128, TILES], f32, kind="ExternalInput")
    rows_r = nc.dram_tensor("rows_r", [4, SH_PAD], bf, kind="ExternalInput")
    # rows_r rows: 0 = sqdeg, 1 = dinv, 2 = dinv^2, 3 = ones
    idx_d = nc.dram_tensor("idx", [128, tot_c * 8], dt.int16, kind="ExternalInput")
    dloc_d = nc.dram_tensor("dloc", [128, tot_c], f32, kind="ExternalInput")
    y = nc.dram_tensor("y", [16, SH], f32, kind="ExternalOutput")

    with tile.TileContext(nc) as tc:
        with tc.tile_pool(name="const", bufs=1) as constp, \
             tc.tile_pool(name="dram", bufs=1, space="DRAM") as dram:
            idx_t = constp.tile([128, tot_c * 8], dt.int16)
            nc.sync.dma_start(out=idx_t[:], in_=idx_d[:])
            dloc_t = constp.tile([128, tot_c], bf)
            nc.gpsimd.dma_start(out=dloc_t[:], in_=dloc_d[:])
            dinv_s = constp.tile([128, TILES], f32)
            nc.sync.dma_start(out=dinv_s[:], in_=dinv_t[:])
            rowpack = constp.tile([65, SH_PAD], bf)
            nc.sync.dma_start(out=rowpack[0:1, :], in_=rows_r[0:1, :])
            nc.sync.dma_start(out=rowpack[32:33, :], in_=rows_r[1:2, :])
            nc.sync.dma_start(out=rowpack[64:65, :], in_=rows_r[2:3, :])
            ones_sm = constp.tile([65, 128], bf)
            nc.vector.memset(ones_sm[:], 1.0)
            brow_s = constp.tile([1, H1], bf)
            nc.sync.dma_start(out=brow_s[:], in_=brow[:])
            bcrow_s = constp.tile([1, H2], bf)
            nc.sync.dma_start(out=bcrow_s[:], in_=bcrow[:])
            w2b_s = constp.tile([H1, H2], bf)
            nc.sync.dma_start(out=w2b_s[:], in_=W2b[:])
            ident = constp.tile([128, 128], bf)
            make_identity(nc, ident[:])
            iota_i = constp.tile([128, 128], dt.int32)
            nc.gpsimd.iota(iota_i[:], pattern=[[1, 128]], base=0,
                           channel_multiplier=0)
            iota_f = constp.tile([128, 128], bf)
            nc.vector.tensor_copy(iota_f[:], iota_i[:])
            ones_c = constp.tile([16, 1], bf)
            nc.vector.memset(ones_c[:], 1.0)

            hs1_sb = constp.tile([128, TILES, H1], bf)   # own shard rows
            nc.vector.memset(hs1_sb[:], 0.0)
            hs2_sb = constp.tile([128, TILES, H2], bf)
            nc.vector.memset(hs2_sb[:], 0.0)
            z_all = constp.tile([16, SH_PAD], bf)
            s_all = constp.tile([1, SH_PAD], bf)
            ls_all = constp.tile([1, SH_PAD], bf)

            hs1_shard = dram.tile([SH_PAD, H1P], bf)
            hs1_full = dram.tile([NT, H1P], bf, addr_space="Shared")
            hs2_shard = dram.tile([SH_PAD, H2P], bf)
            hs2_full = dram.tile([NT, H2P], bf)

            # ---------------- Phase A ----------------
            with tc.tile_pool(name="phA", bufs=3) as pa, \
                 tc.tile_pool(name="phA_w", bufs=1) as paw, \
                 tc.tile_pool(name="psA", bufs=4, space="PSUM") as psa:
                w1_s = paw.tile([128, FX // 128, 64], bf)
                for k in range(FX // 128):
                    nc.sync.dma_start(out=w1_s[:, k, :],
                                      in_=W1[k * 128:(k + 1) * 128, :])
                wd1_s = paw.tile([FD, 32], bf)
                nc.sync.dma_start(out=wd1_s[:], in_=Wd1[:])
                xT_s = []
                for k in range(FX // 128):
                    st = paw.tile([128, SH], bf, tag=f"xT{k}")
                    nc.sync.dma_start(out=st[:],
                                      in_=xT[k * 128:(k + 1) * 128, :])
                    xT_s.append(st)
                datT_s = paw.tile([FD, SH], bf)
                nc.sync.dma_start(out=datT_s[:], in_=datT[:])

                for t in range(TILES):
                    nt = min(128, SH - t * 128)
                    ps = psa.tile([128, H1], f32, space="PSUM", tag="psA")
                    for k in range(FX // 128):
                        nc.tensor.matmul(
                            out=ps[:nt, 0:64],
                            lhsT=xT_s[k][:, t * 128:t * 128 + nt],
                            rhs=w1_s[:, k, :],
                            start=(k == 0), stop=(k == FX // 128 - 1))
                    nc.tensor.matmul(
                        out=ps[:nt, 64:96],
                        lhsT=datT_s[:, t * 128:t * 128 + nt],
                        rhs=wd1_s[:],
                        start=True, stop=True)
                    nc.vector.tensor_tensor(
                        out=hs1_sb[:nt, t, :], in0=ps[:nt, :],
                        in1=dinv_s[:nt, t:t + 1].to_broadcast([nt, H1]),
                        op=mybir.AluOpType.mult)
                nc.sync.dma_start(
                    out=hs1_shard[0:SH_PAD, 0:H1].rearrange(
                        "(t p) f -> p t f", p=128),
                    in_=hs1_sb[:, :, :])
            nc.gpsimd.collective_compute(
                "AllGather", mybir.AluOpType.bypass,
                replica_groups=[list(range(NCORES))],
                ins=[hs1_shard.opt()], outs=[hs1_full.opt()])

            # ---------------- Phase B ----------------
            with tc.tile_pool(name="phB", bufs=8) as pb, \
                 tc.tile_pool(name="phBw", bufs=6) as pbw, \
                 tc.tile_pool(name="psB", bufs=2, space="PSUM") as psb, \
                 tc.tile_pool(name="psBd", bufs=2, space="PSUM") as psbd, \
                 tc.tile_pool(name="psB2", bufs=2, space="PSUM") as psb2:
                off = 0
                for t in range(TILES):
                    ct = C_t[t]
                    nt = min(128, SH - t * 128)
                    ni = ct * 128
                    tr = slice(t * 128, (t + 1) * 128)
                    msg = pb.tile([128, ct, H1P], bf, tag="msg")
                    nc.gpsimd.dma_gather(
                        out_ap=msg[:], in_ap=hs1_full[BIAS:, :],
                        idxs_ap=idx_t[:, off * 8:(off + ct) * 8],
                        num_idxs=ni, num_idxs_reg=ni, elem_size=H1P,
                        single_packet=False, queue_num=t % 4)
                    W = pbw.tile([128, ct, 128], bf, tag="W")
                    nc.any.tensor_tensor(
                        out=W[:],
                        in0=iota_f[:, None, :].to_broadcast([128, ct, 128]),
                        in1=dloc_t[:, off:off + ct, None].to_broadcast(
                            [128, ct, 128]),
                        op=mybir.AluOpType.is_equal)
                    psT = psb.tile([H1, 128], f32, space="PSUM", tag="psT")
                    for j in range(ct):
                        nc.tensor.matmul(
                            out=psT[:], lhsT=msg[:, j, :H1], rhs=W[:, j, :],
                            start=(j == 0), stop=False)
                    # self-loop rows (transpose-accumulate own shard tile)
                    nc.tensor.matmul(
                        out=psT[:], lhsT=hs1_sb[:, t, :], rhs=ident[:],
                        start=False, stop=False)
                    # bias row: aggT += brow^T @ sqdeg_row
                    nc.tensor.matmul(
                        out=psT[:], lhsT=brow_s[:], rhs=rowpack[0:1, tr],
                        start=False, stop=True)
                    # dinv^2 broadcast [H1, 128]
                    psD = psbd.tile([H1, 128], f32, space="PSUM", tag="psD")
                    nc.tensor.matmul(
                        out=psD[:], lhsT=ones_sm[64:65, 0:H1], rhs=rowpack[64:65, tr],
                        start=True, stop=True)
                    h1r = pb.tile([H1, 128], f32, tag="h1r")
                    nc.vector.tensor_scalar(
                        out=h1r[:], in0=psT[:], scalar1=0.0, scalar2=None,
                        op0=mybir.AluOpType.max)
                    h1sT = pb.tile([H1, 128], bf, tag="h1sT")
                    nc.vector.tensor_tensor(
                        out=h1sT[:], in0=h1r[:], in1=psD[:],
                        op=mybir.AluOpType.mult)
                    # layer 2 (branch-summed): hs2 = h1sT^T @ w2b
                    ps2 = psb2.tile([128, H2], f32, space="PSUM", tag="ps2")
                    nc.tensor.matmul(out=ps2[:], lhsT=h1sT[0:64, :],
                                     rhs=w2b_s[0:64, :], start=True, stop=False)
                    nc.tensor.matmul(out=ps2[:], lhsT=h1sT[64:96, :],
                                     rhs=w2b_s[64:96, :], start=False, stop=True)
                    nc.vector.tensor_copy(hs2_sb[:, t, :], ps2[:])
                    nc.sync.dma_start(
                        out=hs2_shard[t * 128:t * 128 + nt, 0:H2],
                        in_=hs2_sb[:nt, t, :])
                    if t == R0 // 128 - 1:
                        nc.gpsimd.collective_compute(
                            "AllGather", mybir.AluOpType.bypass,
                            replica_groups=[list(range(NCORES))],
                            ins=[hs2_shard[0:R0, :].opt()],
                            outs=[hs2_full[0:NCORES * R0, :].opt()])
                    off += ct

            nc.gpsimd.collective_compute(
                "AllGather", mybir.AluOpType.bypass,
                replica_groups=[list(range(NCORES))],
                ins=[hs2_shard[R0:SH_PAD, :].opt()],
                outs=[hs2_full[NCORES * R0:NT, :].opt()])

            # ---------------- Phase C ----------------
            with tc.tile_pool(name="phC", bufs=8) as pc_, \
                 tc.tile_pool(name="phCw", bufs=6) as pcw, \
                 tc.tile_pool(name="psC", bufs=2, space="PSUM") as psc, \
                 tc.tile_pool(name="psCd", bufs=2, space="PSUM") as pscd, \
                 tc.tile_pool(name="psCs", bufs=2, space="PSUM") as pscs:
                off = 0
                for t in range(TILES):
                    ct = C_t[t]
                    nt = min(128, SH - t * 128)
                    ni = ct * 128
                    tr = slice(t * 128, (t + 1) * 128)
                    msg = pc_.tile([128, ct, H2P], bf, tag="msg2")
                    nc.gpsimd.dma_gather(
                        out_ap=msg[:], in_ap=hs2_full[BIAS:, :],
                        idxs_ap=idx_t[:, off * 8:(off + ct) * 8],
                        num_idxs=ni, num_idxs_reg=ni, elem_size=H2P,
                        single_packet=False, queue_num=t % 4)
                    W = pcw.tile([128, ct, 128], bf, tag="W2")
                    nc.any.tensor_tensor(
                        out=W[:],
                        in0=iota_f[:, None, :].to_broadcast([128, ct, 128]),
                        in1=dloc_t[:, off:off + ct, None].to_broadcast(
                            [128, ct, 128]),
                        op=mybir.AluOpType.is_equal)
                    psC = psc.tile([H2, 128], f32, space="PSUM", tag="psC")
                    for j in range(ct):
                        nc.tensor.matmul(
                            out=psC[:], lhsT=msg[:, j, :H2], rhs=W[:, j, :],
                            start=(j == 0), stop=False)
                    nc.tensor.matmul(
                        out=psC[:], lhsT=hs2_sb[:, t, :], rhs=ident[:],
                        start=False, stop=False)
                    nc.tensor.matmul(
                        out=psC[:], lhsT=bcrow_s[:], rhs=rowpack[0:1, tr],
                        start=False, stop=True)
                    psD = pscd.tile([H2, 128], f32, space="PSUM", tag="psDC")
                    nc.tensor.matmul(
                        out=psD[:], lhsT=ones_sm[32:33, 0:H2], rhs=rowpack[32:33, tr],
                        start=True, stop=True)
                    dDC = pc_.tile([H2, 128], f32, tag="dDC")
                    nc.vector.tensor_copy(dDC[:], psD[:])
                    nc.vector.tensor_tensor(
                        out=z_all[:, tr], in0=psC[:], in1=dDC[:],
                        op=mybir.AluOpType.mult)
                    ex = pc_.tile([H2, 128], bf, tag="ex")
                    nc.scalar.activation(
                        ex[:], z_all[:, tr], mybir.ActivationFunctionType.Exp)
                    psS = pscs.tile([1, 128], f32, space="PSUM", tag="psS")
                    nc.tensor.matmul(
                        out=psS[:], lhsT=ones_c[:], rhs=ex[:],
                        start=True, stop=True)
                    nc.vector.tensor_copy(s_all[:, tr], psS[:])
                    if t == R0 // 128 - 1:
                        nc.scalar.activation(
                            ls_all[:, 0:R0], s_all[:, 0:R0],
                            mybir.ActivationFunctionType.Ln)
                        with tc.tile_pool(name="psEa", bufs=2,
                                          space="PSUM") as psea:
                            for g in range(0, R0 - 512, 512):
                                psL = psea.tile([H2, 512], f32, space="PSUM",
                                                tag="psLa")
                                nc.tensor.matmul(
                                    out=psL[:], lhsT=ones_sm[0:1, 0:H2],
                                    rhs=ls_all[0:1, g:g + 512],
                                    start=True, stop=True)
                                ot = pc_.tile([H2, 512], f32, tag="ota")
                                nc.vector.tensor_tensor(
                                    out=ot[:], in0=z_all[:, g:g + 512],
                                    in1=psL[:], op=mybir.AluOpType.subtract)
                                nc.sync.dma_start(out=y[:, g:g + 512],
                                                  in_=ot[:])
                    off += ct

                # batched epilogue: ls = ln(s); out = z - ls
                nc.scalar.activation(
                    ls_all[:, R0:SH_PAD], s_all[:, R0:SH_PAD],
                    mybir.ActivationFunctionType.Ln)
                NGRP = 512
                with tc.tile_pool(name="psE", bufs=2, space="PSUM") as pse:
                    for g in range(R0 - NGRP, SH, NGRP):
                        w = min(NGRP, SH - g)
                        psL = pse.tile([H2, NGRP], f32, space="PSUM", tag="psL")
                        nc.tensor.matmul(
                            out=psL[:, :w], lhsT=ones_sm[0:1, 0:H2],
                            rhs=ls_all[0:1, g:g + w], start=True, stop=True)
                        ot = pc_.tile([H2, NGRP], f32, tag="ot")
                        nc.vector.tensor_tensor(
                            out=ot[:, :w], in0=z_all[:, g:g + w],
                            in1=psL[:, :w], op=mybir.AluOpType.subtract)
                        nc.sync.dma_start(out=y[:, g:g + w], in_=ot[:, :w])

    nc.compile()
    return nc


def kernel(x, dat, edge_index, W1, b1, W2, b2, Wd1, bd1, Wd2, bd2):
    x = np.asarray(x, dtype=np.float32)
    dat = np.asarray(dat, dtype=np.float32)
    dinv, sqdeg, C_t, idx_all, dloc_all = _host_prep(np.asarray(edge_index))

    key = tuple(C_t)
    if key not in _CACHE:
        _CACHE[key] = _build(C_t)
    nc = _CACHE[key]

    W1f = np.asarray(W1, np.float32).astype(BF16)
    Wd1f = np.asarray(Wd1, np.float32).astype(BF16)
    w2b = np.concatenate([0.2 * np.asarray(W2, np.float32),
                          0.1 * np.asarray(Wd2, np.float32)], axis=0).astype(BF16)
    brow = np.concatenate([np.asarray(b1, np.float32),
                           np.asarray(bd1, np.float32)])[None, :].astype(BF16)
    bcrow = (0.2 * np.asarray(b2, np.float32) +
             0.1 * np.asarray(bd2, np.float32))[None, :].astype(BF16)

    in_maps = []
    for c in range(NCORES):
        lo, hi = c * SH, (c + 1) * SH
        dv = np.zeros((128, TILES), np.float32)
        dv_flat = dinv[lo:hi]
        dv[:, :TILES - 1] = dv_flat[:(TILES - 1) * 128].reshape(TILES - 1, 128).T
        rem = SH - (TILES - 1) * 128
        dv[:rem, TILES - 1] = dv_flat[(TILES - 1) * 128:]
        rows4 = np.zeros((4, SH_PAD), np.float32)
        rows4[0, :SH] = sqdeg[lo:hi]
        rows4[1, :SH] = dinv[lo:hi]
        rows4[2, :SH] = dinv[lo:hi] ** 2
        rows4[3, :] = 1.0
        in_maps.append({
            "xT": np.ascontiguousarray(x[lo:hi].T).astype(BF16),
            "datT": np.ascontiguousarray(dat[lo:hi].T).astype(BF16),
            "W1": W1f, "Wd1": Wd1f, "W2b": w2b,
            "brow": brow, "bcrow": bcrow,
            "dinv_t": dv, "rows_r": rows4.astype(BF16),
            "idx": idx_all[c], "dloc": dloc_all[c],
        })

    res = run_bass_kernel_spmd(nc, in_maps, core_ids=list(range(NCORES)))
    out = np.concatenate(
        [np.asarray(res.results[c]["y"]).T for c in range(NCORES)], axis=0)
    return out.astype(np.float32)
